# revision 2
# baseline (speedup 1.0000x reference)
"""Trainium2 Bass kernel for nn_MirrorDescentLinear.

Reference computation:
    w[o,i] = (e1 - e0) / (1 + e0 + e1)            (softmax(+1) - softmax(-1))
    w *= bf16(scales)[o, i//128]                   (per-group scale)
    w *= mask[o,i]                                 (0/1 int mask)
    y = x @ w.T                                    (f32, [8192,4096]@[4096,4096].T)

Sharding (8 cores): tensor-parallel 4-way on out_features x 2-way on tokens.
Each core computes y[t_half, o_quarter] from logits/scales/mask[o_quarter]
and xT[:, t_half]. The host pre-transposes x once (layout-only) so the
contraction dim I lands on SBUF partitions without any on-chip transpose of x.

Per-core device program:
  phase A (weights): exp on ScalarE, arithmetic on VectorE, 1/d as exp(-ln d)
    on ScalarE, mask via copy_predicated, per-128-group scale via
    scalar_tensor_tensor; w tiles transposed on TensorE into a resident
    wT[i, o] (float32r) so phase B can consume per-i-chunk as soon as ready.
  phase B (matmul): float32r matmuls (full bf16 rate at N=512, FP22 mantissa)
    accumulating over 32 i-chunks into PSUM; ScalarE evacuates, DMA stores y.
"""

import json
import sys

sys.path.insert(0, "/opt/trn_rl_repo")

import numpy as np

import concourse.bass as bass
import concourse.tile as tile
from concourse import mybir
from concourse.bass_utils import run_bass_kernel_spmd
from concourse.masks import make_identity
from concourse.tile_scheduler import N_PROCS
from concourse.vector_clock import ScopedClock, VectorClock

# ---------------------------------------------------------------------------
# Compatibility patches for the bundled walrus (accepts at most ONE sync wait
# per instruction; rejects any wait on Drain).
# ---------------------------------------------------------------------------


def _drain_and_barrier_split(self, tick_clock, wait_clock):
    g = tick_clock.global_clock
    for p in range(N_PROCS):
        tick = g.peek_next(p) - 1
        if tick <= 0:
            continue
        vc = VectorClock()
        vc.require_at_least(p, tick)
        nop = self.nc.sync.nop(nofuse=True, hint="tail_wait_split")
        wait_clock.add_sem_waits(nop.ins, ScopedClock({None: vc}))

    self.nc.sync.drain()

    self.nc.all_engine_barrier()
    assert self.sems is not None
    popped = self.nc._tile_sem_poison_stack.pop()
    assert popped is self._sem_poison
    self.nc.clear_and_free_semaphores(list(self.sems.allocated().values()))
    self.nc.all_engine_barrier()


_orig_to_json_bytes = bass.Bass.to_json_bytes
_split_ctr = [0]


def _to_json_bytes_split(self):
    raw = _orig_to_json_bytes(self)
    m = json.loads(raw)
    changed = False
    for fn in m.get("functions", []):
        for blk in fn.get("blocks", []):
            insts = blk.get("instructions")
            if not insts:
                continue
            out = []
            for inst in insts:
                si = inst.get("sync_info")
                ow = (si or {}).get("on_wait") or []
                eng = inst.get("engine")
                if len(ow) > 1 and eng:
                    changed = True
                    for w in ow[:-1]:
                        _split_ctr[0] += 1
                        nop = {
                            "engine": eng,
                            "ins": [],
                            "outs": [],
                            "name": f"I-wsplit-{_split_ctr[0]}",
                            "opcode": "NoOp",
                            "sync_info": {"on_update": [], "on_wait": [w]},
                            "text_hint": "wait_split",
                        }
                        if inst.get("debug") is not None:
                            nop["debug"] = inst["debug"]
                        out.append(nop)
                    si["on_wait"] = [ow[-1]]
                out.append(inst)
            blk["instructions"] = out
    return json.dumps(m).encode() if changed else raw


_patched = False


def _install_patches():
    global _patched
    if _patched:
        return
    tile.TileContext._drain_and_barrier = _drain_and_barrier_split
    bass.Bass.to_json_bytes = _to_json_bytes_split
    _patched = True


# ---------------------------------------------------------------------------
# Problem constants (hardcoded per contest rules)
# ---------------------------------------------------------------------------

T_FULL, O_FULL, I_FULL, G = 8192, 4096, 4096, 128
N_OSH, N_TSH = 4, 2  # o-quarters x t-halves = 8 cores
O_SH, T_SH = O_FULL // N_OSH, T_FULL // N_TSH  # 1024, 4096
NK = I_FULL // 128  # 32 contraction chunks of 128
N_IC = 8  # i-chunks of 512 in phase A
N_OB = O_SH // 128  # 8 o-blocks per core
N_TT = T_SH // 128  # 32 t-tiles per core

f32 = mybir.dt.float32
f32r = mybir.dt.float32r
i32 = mybir.dt.int32
bf16 = mybir.dt.bfloat16

AF = mybir.ActivationFunctionType
ALU = mybir.AluOpType


def build_program() -> bass.Bass:
    _install_patches()
    nc = bass.Bass()
    xT = nc.declare_dram_parameter("xT", [I_FULL, T_SH], f32r, isOutput=False)
    logits = nc.declare_dram_parameter("logits", [O_SH, I_FULL, 2], f32, isOutput=False)
    scales = nc.declare_dram_parameter("scales", [O_SH, I_FULL // G], f32, isOutput=False)
    mask = nc.declare_dram_parameter("mask", [O_SH, I_FULL], i32, isOutput=False)
    y = nc.declare_dram_parameter("y", [T_SH, O_SH], f32, isOutput=True)

    xT_t = xT.rearrange("(k p) t -> p k t", p=128)  # [128, NK, T_SH]
    scales_t = scales.rearrange("(ob p) g -> p ob g", p=128)  # [128, N_OB, 32]

    with tile.TileContext(nc) as tc:
        with (
            tc.tile_pool(name="persist", bufs=1) as persist,
            tc.tile_pool(name="wt", bufs=1) as wt_pool,
            tc.tile_pool(name="wa", bufs=2) as wa,
            tc.tile_pool(name="xin", bufs=2) as xin,
            tc.tile_pool(name="yout", bufs=2) as yout,
            tc.tile_pool(name="psa", bufs=4, space="PSUM") as psa,
            tc.tile_pool(name="psb", bufs=4, space="PSUM") as psb,
        ):
            ident = persist.tile([128, 128], f32)
            make_identity(nc, ident)

            # scales for all o-blocks, rounded through bf16 once
            s_raw = persist.tile([128, N_OB, 32], f32, tag="sraw")
            nc.sync.dma_start(out=s_raw, in_=scales_t)
            s_bf = persist.tile([128, N_OB, 32], bf16, tag="sbf")
            nc.vector.tensor_copy(out=s_bf, in_=s_raw)
            s_r = persist.tile([128, N_OB, 32], f32, tag="sr")
            nc.vector.tensor_copy(out=s_r, in_=s_bf)

            # resident transposed weights, one tile per 128-contraction chunk
            wT = [
                wt_pool.tile([128, O_SH], f32r, tag=f"wT{k}", name=f"wT{k}")
                for k in range(NK)
            ]

            # ---- phase A: weights (ic-outer so wT[k] complete early) ----
            for ic in range(N_IC):
                for ob in range(N_OB):
                    L = wa.tile([128, 512, 2], f32, tag="L")
                    nc.sync.dma_start(
                        out=L, in_=logits[ob * 128 : (ob + 1) * 128, ic * 512 : (ic + 1) * 512, :]
                    )
                    M = wa.tile([128, 512], i32, tag="M")
                    nc.sync.dma_start(
                        out=M, in_=mask[ob * 128 : (ob + 1) * 128, ic * 512 : (ic + 1) * 512]
                    )
                    # E = exp(logits), in place
                    Lf = L.rearrange("p i s -> p (i s)")
                    nc.scalar.activation(out=Lf, in_=Lf, func=AF.Exp)
                    # D = e0 + 1 + e1
                    D = wa.tile([128, 512], f32, tag="D")
                    nc.vector.scalar_tensor_tensor(
                        out=D, in0=L[:, :, 0], scalar=1.0, in1=L[:, :, 1],
                        op0=ALU.add, op1=ALU.add,
                    )
                    # R = 1/D = exp(-ln D)   (ScalarE; DVE reciprocal is slow)
                    nc.scalar.activation(out=D, in_=D, func=AF.Ln)
                    R = wa.tile([128, 512], f32, tag="R")
                    nc.scalar.activation(out=R, in_=D, func=AF.Exp, scale=-1.0)
                    # N = e1 - e0
                    N = wa.tile([128, 512], f32, tag="N")
                    nc.vector.tensor_tensor(
                        out=N, in0=L[:, :, 1], in1=L[:, :, 0], op=ALU.subtract
                    )
                    # W = (N * s_g) * R  per 128-wide scale group
                    W = wa.tile([128, 512], f32, tag="W")
                    for g in range(4):
                        sl = slice(g * 128, (g + 1) * 128)
                        ga = ic * 4 + g
                        nc.vector.scalar_tensor_tensor(
                            out=W[:, sl], in0=N[:, sl],
                            scalar=s_r[:, ob, ga : ga + 1],
                            in1=R[:, sl], op0=ALU.mult, op1=ALU.mult,
                        )
                    # apply mask: Wf = W where mask else 0
                    Wf = wa.tile([128, 512], f32, tag="Wf")
                    nc.gpsimd.memset(Wf, 0.0)
                    nc.vector.copy_predicated(out=Wf, mask=M, data=W)
                    # transpose 128x128 blocks into wT[k][:, ob*128:...]
                    for q in range(4):
                        k = ic * 4 + q
                        pt = psa.tile([128, 128], f32, tag="pt")
                        nc.tensor.transpose(
                            out=pt, in_=Wf[:, q * 128 : (q + 1) * 128], identity=ident
                        )
                        nc.scalar.copy(
                            out=wT[k][:, ob * 128 : (ob + 1) * 128], in_=pt
                        )

            # ---- phase B: y[t, o] = sum_k xT[k,t].T @ wT[k][:, o] ----
            for tt in range(N_TT):
                xTt = xin.tile([128, NK, 128], f32r, tag="xTt")
                nc.sync.dma_start(
                    out=xTt, in_=xT_t[:, :, tt * 128 : (tt + 1) * 128]
                )
                pb0 = psb.tile([128, 512], f32, tag="pb")
                pb1 = psb.tile([128, 512], f32, tag="pb")
                pbs = [pb0, pb1]
                for k in range(NK):
                    for oc in range(2):
                        nc.tensor.matmul(
                            out=pbs[oc],
                            lhsT=xTt[:, k, :],
                            rhs=wT[k][:, oc * 512 : (oc + 1) * 512],
                            start=(k == 0),
                            stop=(k == NK - 1),
                        )
                y_sb = yout.tile([128, O_SH], f32, tag="ysb")
                nc.scalar.copy(out=y_sb[:, 0:512], in_=pb0)
                nc.scalar.copy(out=y_sb[:, 512:1024], in_=pb1)
                nc.sync.dma_start(
                    out=y[tt * 128 : (tt + 1) * 128, :], in_=y_sb
                )

    return nc


_prog = None


def _get_program() -> bass.Bass:
    global _prog
    if _prog is None:
        _prog = build_program()
    return _prog


def kernel(x, logits, scales, mask):
    nc = _get_program()
    x = np.asarray(x, dtype=np.float32)
    logits = np.asarray(logits, dtype=np.float32)
    scales = np.asarray(scales, dtype=np.float32)
    mask_i = np.asarray(mask, dtype=np.int32)

    xT = np.ascontiguousarray(x.T)  # [I, T]
    in_maps = []
    for c in range(8):
        th, oq = divmod(c, N_OSH)
        in_maps.append(
            {
                "xT": np.ascontiguousarray(xT[:, th * T_SH : (th + 1) * T_SH]),
                "logits": np.ascontiguousarray(logits[oq * O_SH : (oq + 1) * O_SH]),
                "scales": np.ascontiguousarray(scales[oq * O_SH : (oq + 1) * O_SH]),
                "mask": np.ascontiguousarray(mask_i[oq * O_SH : (oq + 1) * O_SH]),
            }
        )
    res = run_bass_kernel_spmd(nc, in_maps, core_ids=list(range(8)))
    yf = np.empty((T_FULL, O_FULL), dtype=np.float32)
    for c in range(8):
        th, oq = divmod(c, N_OSH)
        yf[th * T_SH : (th + 1) * T_SH, oq * O_SH : (oq + 1) * O_SH] = res.results[c][
            "y"
        ]
    return yf
